# revision 6
# baseline (speedup 1.0000x reference)
"""Trainium2 Bass kernel for nn_MeshEdgeBlock (GNN edge-block message passing).

Computes, per edge e with endpoints (s, d):
    x  = concat([src_nodes[s], dst_nodes[d], edge_feat[e]])   # [384]
    h  = silu(x @ W1 + b1)                                    # [512]
    y  = h @ W2 + b2                                          # [128]
    y  = LayerNorm(y) * gamma + beta + edge_feat[e]           # [128]

Sharding: edges (and index arrays) split evenly across the 8 NeuronCores;
node-feature tables and MLP weights replicated to every core.

Device-side dataflow per core (EC = 31744 padded edges, 62 blocks of
EB = 512 edges = 4 tiles of 128):
  - per block, 8 indirect-DMA gathers (one [128,1]-index column each) pull
    src+dst node rows (bf16) from a host-concatenated [2N, 128] table
    (dst indices offset by N). SWDGE descriptor generation is the pacing
    resource (~6.6 ns/row, locality-insensitive), so everything else is
    structured to hide under it.
  - s/d tiles transposed on TensorE (identity matmul) into feature-major
    layout; edge features arrive pre-transposed from the host (edgesT), so
    only 8 of 12 tile transposes run on device.
  - mm1 streams N=512 edge columns per matmul with W1 chunks stationary:
    12 matmuls per block (vs 48 at N=128), amortizing LDWEIGHTS. PSUM
    [128,512] f32 accumulates over the 3 input chunks per hidden chunk.
  - silu on ScalarE per hidden chunk (b1 == 0 fast path)
  - mm2 per 128-edge tile: 4 matmuls, hT chunk stationary -> y psum (fp32)
  - LN stats per tile via bn_stats/bn_aggr (VectorE); y copied to SBUF by
    ScalarE (Copy shares Silu's activation table set - no table reloads)
  - rsqrt(var+eps) once per block on VectorE (exponent-bit seed + 2 Newton
    steps); normalize + residual fused into one custom-DVE affine_then_add.

Numerics: matmul inputs bf16 (fp32 PSUM accumulation); LN + residual fp32
except edge features, which stay bf16 end-to-end. b1/b2/gamma/beta get a
fast path when they hold the trivial values hardcoded by the problem's
setup_inputs (zeros/ones) - verified on the host per call; non-trivial
values take extra (exact, slightly slower) ops.
"""

import numpy as np
import ml_dtypes
from contextlib import ExitStack

import concourse.bass as bass
import concourse.tile as tile
from concourse import bacc, mybir
from concourse.bass import IndirectOffsetOnAxis
from concourse.bass_utils import run_bass_kernel_spmd
from concourse.masks import make_identity

N_CORES = 8
E_FULL = 250000
N_NODES = 100000
D = 128
H = 512
LN_EPS = 1e-5

G = 4              # tiles per block
EB = G * 128       # 512 edges per block
EC = 31744         # padded edges per core = 248*128, 248 = 62*G
NT = EC // 128     # 248
NB = NT // G       # 62 blocks per core

BF16 = mybir.dt.bfloat16
F32 = mybir.dt.float32
I32 = mybir.dt.int32

RSQRT_MAGIC = 0x5F3759DF

TP_MODE = "pe"     # 'pe' | 'dma'
VARIANT = "full"   # 'full' | 'nogather' | 'gatheronly'
IO_BUFS = 5
PRELOAD_IDX = True    # all idx tiles loaded once at start (2KB/partition),
                      # host-packed as [128, NB*2G] so the load is one
                      # contiguous DMA and gathers never wait on idx DMAs

_PROGRAM_CACHE = {}


def _rsqrt_batched(nc, stats, mg2):
    """inv = rsqrt(var + eps), nmi = -mu * inv over [128, G] tiles."""
    mu = mg2[:, 0:2 * G:2]
    var = mg2[:, 1:2 * G:2]
    veps = stats.tile([128, G], F32, tag="veps")
    nc.vector.tensor_scalar(out=veps[:], in0=var, scalar1=LN_EPS, scalar2=None,
                            op0=mybir.AluOpType.add)
    hv = stats.tile([128, G], F32, tag="hv")
    nc.vector.tensor_scalar(out=hv[:], in0=veps[:], scalar1=-0.5, scalar2=None,
                            op0=mybir.AluOpType.mult)
    sh = stats.tile([128, G], I32, tag="sh")
    nc.vector.tensor_scalar(out=sh[:], in0=veps[:].bitcast(I32), scalar1=1,
                            scalar2=None,
                            op0=mybir.AluOpType.arith_shift_right)
    seed = stats.tile([128, G], I32, tag="seed")
    nc.vector.tensor_scalar(out=seed[:], in0=sh[:], scalar1=-1,
                            scalar2=RSQRT_MAGIC,
                            op0=mybir.AluOpType.mult,
                            op1=mybir.AluOpType.add)
    y = seed[:].bitcast(F32)
    for it in range(2):
        a = stats.tile([128, G], F32, tag=f"nr_a{it}")
        nc.vector.tensor_mul(out=a[:], in0=y, in1=y)
        b = stats.tile([128, G], F32, tag=f"nr_b{it}")
        nc.vector.tensor_mul(out=b[:], in0=a[:], in1=hv[:])
        ynew = stats.tile([128, G], F32, tag=f"nr_y{it}")
        nc.vector.scalar_tensor_tensor(out=ynew[:], in0=b[:], scalar=1.5,
                                       in1=y, op0=mybir.AluOpType.add,
                                       op1=mybir.AluOpType.mult)
        y = ynew[:]
    nmi = stats.tile([128, G], F32, tag="nmi")
    nc.vector.scalar_tensor_tensor(out=nmi[:], in0=mu, scalar=-1.0, in1=y,
                                   op0=mybir.AluOpType.mult,
                                   op1=mybir.AluOpType.mult)
    return y, nmi


def _build_program(trivial_affine: bool, repeats: int = 1):
    key = (trivial_affine, TP_MODE, repeats, VARIANT)
    if key in _PROGRAM_CACHE:
        return _PROGRAM_CACHE[key]
    do_gather = VARIANT in ("full", "gatheronly")
    do_compute = VARIANT in ("full", "nogather")

    nc = bacc.Bacc("TRN2", target_bir_lowering=False, debug=False,
                   num_devices=N_CORES)

    nodes = nc.dram_tensor("nodes", [2 * N_NODES, D], BF16, kind="ExternalInput").ap()
    edges = nc.dram_tensor("edges", [EC, D], BF16, kind="ExternalInput").ap()
    edgesT = nc.dram_tensor("edgesT", [D, EC], BF16, kind="ExternalInput").ap()
    idx = nc.dram_tensor("idx", [128, NB * 2 * G], I32, kind="ExternalInput").ap()
    w1 = nc.dram_tensor("w1", [D, 12 * D], BF16, kind="ExternalInput").ap()
    w2 = nc.dram_tensor("w2", [D, 4 * D], BF16, kind="ExternalInput").ap()
    out = nc.dram_tensor("out", [EC, D], F32, kind="ExternalOutput").ap()
    scratch = None
    if VARIANT == "gatheronly":
        scratch = nc.dram_tensor("scratch", [2 * EB, D], BF16).ap()
    if not trivial_affine:
        b1d = nc.dram_tensor("b1d", [D, 4], F32, kind="ExternalInput").ap()
        b2d = nc.dram_tensor("b2d", [D, D], F32, kind="ExternalInput").ap()
        gmd = nc.dram_tensor("gmd", [D, D], F32, kind="ExternalInput").ap()
        btd = nc.dram_tensor("btd", [D, D], F32, kind="ExternalInput").ap()

    with tile.TileContext(nc) as tc, ExitStack() as ctx:
        const = ctx.enter_context(tc.tile_pool(name="const", bufs=1))
        io = ctx.enter_context(tc.tile_pool(name="io", bufs=IO_BUFS))
        idxp = ctx.enter_context(tc.tile_pool(name="idx", bufs=4))
        xtp = ctx.enter_context(tc.tile_pool(name="xt", bufs=3))
        htp = ctx.enter_context(tc.tile_pool(name="ht", bufs=2))
        stats = ctx.enter_context(tc.tile_pool(name="stats", bufs=2))
        ps_h = ctx.enter_context(tc.tile_pool(name="ps_h", bufs=2, space="PSUM"))
        ps_y = ctx.enter_context(tc.tile_pool(name="ps_y", bufs=2, space="PSUM"))
        if TP_MODE == "pe":
            ps_tp = ctx.enter_context(tc.tile_pool(name="ps_tp", bufs=2, space="PSUM"))

        w1sb = const.tile([D, 12 * D], BF16)
        nc.sync.dma_start(out=w1sb[:], in_=w1[:])
        w2sb = const.tile([D, 4 * D], BF16)
        nc.sync.dma_start(out=w2sb[:], in_=w2[:])
        if TP_MODE == "pe":
            ident = const.tile([D, D], BF16)
            make_identity(nc, ident[:])
        if not trivial_affine:
            b1sb = const.tile([D, 4], F32)
            nc.sync.dma_start(out=b1sb[:], in_=b1d[:])
            b2sb = const.tile([D, D], F32)
            nc.sync.dma_start(out=b2sb[:], in_=b2d[:])
            gmsb = const.tile([D, D], F32)
            nc.sync.dma_start(out=gmsb[:], in_=gmd[:])
            btsb = const.tile([D, D], F32)
            nc.sync.dma_start(out=btsb[:], in_=btd[:])

        it_all = const.tile([128, NB, 2 * G], I32)
        nc.sync.dma_start(
            out=it_all[:],
            in_=idx[:].rearrange("p (b k) -> p b k", b=NB))

        def _block(b):
            base = b * EB
            it_ = it_all[:, b, :]
            sd = io.tile([128, 2 * G, D], BF16, tag="sd")
            if do_gather:
                for k in range(2 * G):
                    nc.gpsimd.indirect_dma_start(
                        out=sd[:, k, :], out_offset=None, in_=nodes[:],
                        in_offset=IndirectOffsetOnAxis(ap=it_[:, k:k + 1], axis=0))
            else:
                b2_ = (b * 2 * EB) % (2 * N_NODES - 2 * EB)
                nc.sync.dma_start(
                    out=sd[:],
                    in_=nodes[b2_:b2_ + 2 * EB, :].rearrange(
                        "(p k) f -> p k f", k=2 * G))
            e_bf = io.tile([128, G, D], BF16, tag="e")
            nc.sync.dma_start(
                out=e_bf[:],
                in_=edges[base:base + EB, :].rearrange("(p g) f -> p g f", g=G))

            if not do_compute:
                nc.sync.dma_start(
                    out=scratch[:].rearrange("(p k) f -> p k f", k=2 * G),
                    in_=sd[:])
                return

            xt_e = xtp.tile([128, EB], BF16, tag="xte")
            nc.sync.dma_start(out=xt_e[:], in_=edgesT[:, base:base + EB])

            # transpose gathered s/d tiles -> xt_sd [128 feat, 2, EB edges]
            xt_sd = xtp.tile([128, 2, EB], BF16, tag="xtsd")
            if TP_MODE == "pe":
                tp = ps_tp.tile([128, 2 * EB], BF16)
                for t in range(G):
                    nc.tensor.transpose(out=tp[:, t * D:(t + 1) * D],
                                        in_=sd[:, 2 * t, :], identity=ident[:])
                    nc.tensor.transpose(out=tp[:, EB + t * D:EB + (t + 1) * D],
                                        in_=sd[:, 2 * t + 1, :], identity=ident[:])
                nc.vector.tensor_copy(out=xt_sd[:], in_=tp[:])
            else:
                for t in range(G):
                    nc.sync.dma_start(out=xt_sd[:, 0, t * D:(t + 1) * D],
                                      in_=sd[:, 2 * t, :], transpose=True)
                    nc.sync.dma_start(out=xt_sd[:, 1, t * D:(t + 1) * D],
                                      in_=sd[:, 2 * t + 1, :], transpose=True)

            xs = (xt_sd[:, 0, :], xt_sd[:, 1, :], xt_e[:])

            # mm1: hT chunk m = sum_c W1[c,m]^T @ xT_c, N=EB streams
            ht = htp.tile([128, 4, EB], BF16, tag="ht")
            for m in range(4):
                hps = ps_h.tile([128, EB], F32)
                for c in range(3):
                    nc.tensor.matmul(
                        out=hps[:],
                        lhsT=w1sb[:, (c * 4 + m) * D:(c * 4 + m + 1) * D],
                        rhs=xs[c], start=(c == 0), stop=(c == 2))
                if trivial_affine:
                    nc.scalar.activation(out=ht[:, m, :], in_=hps[:],
                                         func=mybir.ActivationFunctionType.Silu)
                else:
                    nc.scalar.activation(out=ht[:, m, :], in_=hps[:],
                                         func=mybir.ActivationFunctionType.Silu,
                                         bias=b1sb[:, m:m + 1])

            # mm2: per 128-edge tile, y[t] = sum_m ht_m[:, t]^T @ W2_m
            ysb = io.tile([128, G, D], F32, tag="ysb")
            yout = io.tile([128, G, D], F32, tag="yout")
            mg2 = stats.tile([128, 2 * G], F32, tag="mg2")
            yps = ps_y.tile([128, G, D], F32)
            for t in range(G):
                for m in range(4):
                    nc.tensor.matmul(
                        out=yps[:, t, :],
                        lhsT=ht[:, m, t * D:(t + 1) * D],
                        rhs=w2sb[:, m * D:(m + 1) * D],
                        start=(m == 0), stop=(m == 3))
                if not trivial_affine:
                    nc.vector.tensor_add(out=ysb[:, t, :], in0=yps[:, t, :],
                                         in1=b2sb[:])
                else:
                    nc.scalar.activation(out=ysb[:, t, :], in_=yps[:, t, :],
                                         func=mybir.ActivationFunctionType.Copy)
                st6 = stats.tile([128, 6], F32, tag="st6")
                nc.vector.bn_stats(out=st6[:], in_=ysb[:, t, :])
                nc.vector.bn_aggr(out=mg2[:, 2 * t:2 * t + 2], in_=st6[:])

            inv, nmi = _rsqrt_batched(nc, stats, mg2)
            for t in range(G):
                if trivial_affine:
                    nc.vector.affine_then_add(
                        out=yout[:, t, :], in0=ysb[:, t, :], in1=e_bf[:, t, :],
                        scale=inv[:, t:t + 1], bias=nmi[:, t:t + 1])
                else:
                    yn = io.tile([128, D], F32, tag="yn")
                    nc.vector.tensor_scalar(out=yn[:], in0=ysb[:, t, :],
                                            scalar1=inv[:, t:t + 1],
                                            scalar2=nmi[:, t:t + 1],
                                            op0=mybir.AluOpType.mult,
                                            op1=mybir.AluOpType.add)
                    nc.vector.tensor_mul(out=yn[:], in0=yn[:], in1=gmsb[:])
                    nc.vector.tensor_add(out=yn[:], in0=yn[:], in1=btsb[:])
                    nc.vector.tensor_add(out=yout[:, t, :], in0=yn[:],
                                         in1=e_bf[:, t, :])

            nc.sync.dma_start(
                out=out[base:base + EB, :].rearrange("(p g) f -> p g f", g=G),
                in_=yout[:])

        if repeats == 1:
            for b in range(NB):
                _block(b)
        else:
            with tc.For_i(0, repeats, 1):
                for b in range(NB):
                    _block(b)

    nc.compile()
    _PROGRAM_CACHE[key] = nc
    return nc


def _prep(inputs):
    f = {k: np.asarray(v) for k, v in inputs.items()}
    bf = ml_dtypes.bfloat16

    nodes = np.concatenate([f["src_node_features"], f["dst_node_features"]],
                           axis=0).astype(bf)

    e = f["edge_features"].astype(np.float32)
    si = f["src_indices"].astype(np.int64)
    di = f["dst_indices"].astype(np.int64)
    E = e.shape[0]
    etot = EC * N_CORES
    e_pad = np.zeros((etot, D), np.float32)
    e_pad[:E] = e
    idx_pad = np.zeros((etot, 2), np.int32)
    idx_pad[:E, 0] = si.astype(np.int32)
    idx_pad[:E, 1] = (di + N_NODES).astype(np.int32)
    idx_pad[E:, 1] = N_NODES
    e_bf = e_pad.astype(bf)

    W1 = f["W1"].astype(np.float32)
    W2 = f["W2"].astype(np.float32)
    w1b = np.concatenate(
        [W1[c * D:(c + 1) * D, m * D:(m + 1) * D] for c in range(3) for m in range(4)],
        axis=1).astype(bf)
    w2b = np.concatenate([W2[m * D:(m + 1) * D, :] for m in range(4)], axis=1).astype(bf)

    b1 = f["b1"].astype(np.float32)
    b2 = f["b2"].astype(np.float32)
    gm = f["ln_gamma"].astype(np.float32)
    bt = f["ln_beta"].astype(np.float32)
    trivial = (not b1.any()) and (not b2.any()) and (not bt.any()) and bool(np.all(gm == 1.0))

    in_maps = []
    for i in range(N_CORES):
        lo, hi = i * EC, (i + 1) * EC
        ec = e_bf[lo:hi]                       # [EC, D]
        # eT with block-local column order j = g*128 + p  <->  edge p*G + g
        et = np.ascontiguousarray(
            ec.reshape(NB, 128, G, D).transpose(3, 0, 2, 1).reshape(D, EC))
        m = {
            "nodes": nodes,
            "edges": np.ascontiguousarray(ec),
            "edgesT": et,
            # packed [p, b*2G + 2g+c] = idx_pad[lo + b*512 + p*G + g, c]:
            # one contiguous [128, NB*2G] preload, sliced per block on-chip
            "idx": np.ascontiguousarray(
                idx_pad[lo:hi].reshape(NB, 128, G, 2)
                .transpose(1, 0, 2, 3).reshape(128, NB * 2 * G)),
            "w1": w1b,
            "w2": w2b,
        }
        if not trivial:
            m["b1d"] = np.ascontiguousarray(b1.reshape(4, D).T.astype(np.float32))
            m["b2d"] = np.broadcast_to(b2, (D, D)).copy()
            m["gmd"] = np.broadcast_to(gm, (D, D)).copy()
            m["btd"] = np.broadcast_to(bt, (D, D)).copy()
        in_maps.append(m)
    return in_maps, trivial, E


def kernel(**inputs) -> np.ndarray:
    in_maps, trivial, E = _prep(inputs)
    nc = _build_program(trivial)
    res = run_bass_kernel_spmd(nc, in_maps, core_ids=list(range(N_CORES)))
    out = np.concatenate([res.results[i]["out"] for i in range(N_CORES)], axis=0)
    return np.ascontiguousarray(out[:E])
